# revision 6
# baseline (speedup 1.0000x reference)
"""Trainium2 Bass kernel for ExemplarGNN2AdjModel (gnn_message_passing).

Math:
  h  = relu(relu(x@W1+b1)@W2+b2)                      # [512,128] node encoder
  scores[i,j] = Wp2 . relu(Wp1a.h_i + Wp1b.h_j + Wp1c.|h_i-h_j| + bp1) + bp2

Device algorithm (per core, SPMD over 8 cores; core c handles 64 rows of i):
  - Each core receives x pre-rolled by c*64 rows and pre-transposed (xT), so the
    identical program computes rows [c*64, c*64+64) in its local (rolled) node
    order; the host un-rolls the output columns afterwards.
  - |h_i-h_j| = h_i + h_j - 2*min(h_i,h_j): the h_i term is folded into the
    per-i bias (a2 += w3 fold), the h_j term into the B matmul (w2p += w3),
    and the per-pair part is -2*w3^T min(h_i, h_j).
  - Encoder runs on-device in bf16, fp32 PSUM accumulation.  The input DMAs
    are split per k-tile across four DGE rings (gpsimd/vector/scalar/sync) so
    encoder matmuls start as soon as their k-tile lands instead of waiting for
    one monolithic transfer.  Encoder relus are split across ACT and DVE.
  - hT (per-row min scalars) and a2 (per-row relu bias) are only needed for
    this core's 64 local rows -> computed as [128,64] slices, not [128,512].
  - Per row i (64 iters):
      d_i  = min(h, h_i)               (DVE tensor_scalar, pipelined ahead)
      P    = w2p^T h + w3^T d_i        (accumulating PE matmuls into PSUM)
      hid  = relu(P + a2[:,i])         (split: ACT cols [0:SPLIT), DVE rest)
      out[i,:] += emb[:,i,:]^T hid     (PE matmul, Wp2 embedded in col i of a
                                        [128,32] zero matrix -> accumulates row
                                        i into a [32,512] PSUM chunk; deferred
                                        DEFER rows so the PE never waits on the
                                        relus)
  - Output is produced in two [32,512] chunks: chunk A's bias-add + DMA-out
    run mid-loop (overlapped), chunk B's in the tail.
"""

import numpy as np
import ml_dtypes

B = 512
IN_DIM = 595
HID = 128
NCORES = 8
RPC = B // NCORES  # rows per core = 64
HR = RPC // 2      # 32: out-chunk size (M of the out matmuls)
SPLIT = 416        # relu column split: ACT does [0:SPLIT), DVE does [SPLIT:B)
DEFER = 4          # iterations between producing hid(i) and its out-row matmul
G = 2              # rows per stationary-sharing group in the pair loop

# in_dim k-tiles for the first encoder matmul (contraction over 595, padded)
NKT = 5

_PROGRAM_CACHE = {}


def _build_program():
    import concourse.mybir as mybir
    import concourse.tile as tile
    from concourse import bacc

    f32 = mybir.dt.float32
    bf16 = mybir.dt.bfloat16
    Act = mybir.ActivationFunctionType
    Alu = mybir.AluOpType

    nc = bacc.Bacc("TRN2", target_bir_lowering=False)

    # Inputs:
    #   xt  padded to [5*128, 512] -> viewed [128, 5*512] (k-tiles on free dim)
    #   w1  padded to [5*128, 128] -> viewed [128, 5*128]
    #   wpack = [w2 | wp1a | w2p | w3]   [128, 4*128] bf16
    #   bias  = [b1 | b2 | bp1 | bp2col] [128, 4] f32
    #   emb   = two 32-row chunks of the Wp2 diagonal embedding [128, 2*32*32]
    xt_d = nc.dram_tensor("xtp", [HID, NKT * B], bf16, kind="ExternalInput")
    w1_d = nc.dram_tensor("w1p", [HID, NKT * HID], bf16, kind="ExternalInput")
    wpack_d = nc.dram_tensor("wpack", [HID, 4 * HID], bf16, kind="ExternalInput")
    bias_d = nc.dram_tensor("biases", [HID, 4], f32, kind="ExternalInput")
    emb_d = nc.dram_tensor("emb", [HID, 2 * HR * HR], bf16, kind="ExternalInput")
    out_d = nc.dram_tensor("out", [RPC, B], f32, kind="ExternalOutput")

    with tile.TileContext(nc) as tc:
        with (
            tc.tile_pool(name="consts", bufs=1) as consts,
            tc.tile_pool(name="setup", bufs=1) as setup,
            tc.tile_pool(name="work", bufs=9) as work,
            tc.tile_pool(name="penc", bufs=1, space="PSUM") as penc,
            tc.tile_pool(name="ppair", bufs=5, space="PSUM") as ppair,
            tc.tile_pool(name="pout", bufs=1, space="PSUM") as pout,
        ):
            # ---- ACT table preload: a tiny relu at t=0 triggers the lazy
            # ACT_TABLE_LOAD (~1.3us) while the input DMAs are in flight.
            tiny = setup.tile([HID, 1], f32)
            nc.vector.memset(tiny, 0.0)
            tiny2 = setup.tile([HID, 1], f32)
            nc.scalar.activation(tiny2, tiny, Act.Relu)

            # ---- input DMAs: k-tiles split across the three DGE rings
            # (gpsimd/scalar/sync) so the encoder can start after ~one
            # tile's transfer time.
            xt_all = consts.tile([HID, NKT * B], bf16)
            w1_all = consts.tile([HID, NKT * HID], bf16)
            wpack = consts.tile([HID, 4 * HID], bf16)
            biases = consts.tile([HID, 4], f32)
            emb_sb = consts.tile([HID, 2 * HR * HR], bf16)

            def xt_dma(eng, k):
                eng.dma_start(
                    out=xt_all[:, k * B : (k + 1) * B],
                    in_=xt_d[:, k * B : (k + 1) * B],
                )

            xt_dma(nc.gpsimd, 0)
            xt_dma(nc.gpsimd, 1)
            nc.scalar.dma_start(out=w1_all[:, 0:HID], in_=w1_d[:, 0:HID])
            xt_dma(nc.gpsimd, 2)
            nc.scalar.dma_start(
                out=w1_all[:, HID : NKT * HID], in_=w1_d[:, HID : NKT * HID]
            )
            xt_dma(nc.sync, 3)
            xt_dma(nc.sync, 4)
            nc.scalar.dma_start(out=wpack, in_=wpack_d[:, :])
            nc.gpsimd.dma_start(out=biases, in_=bias_d[:, :])
            nc.sync.dma_start(out=emb_sb, in_=emb_d[:, :])

            xt_sb = [xt_all[:, k * B : (k + 1) * B] for k in range(NKT)]
            w1_sb = [w1_all[:, k * HID : (k + 1) * HID] for k in range(NKT)]
            w2_sb = wpack[:, 0 * HID : 1 * HID]
            wp1a_sb = wpack[:, 1 * HID : 2 * HID]
            w2p_sb = wpack[:, 2 * HID : 3 * HID]
            w3_sb = wpack[:, 3 * HID : 4 * HID]
            b1_sb = biases[:, 0:1]
            b2_sb = biases[:, 1:2]
            bp1_sb = biases[:, 2:3]
            bp2a_sb = biases[0:HR, 3:4]

            # ---- encoder: h1 = relu(W1^T xT + b1), hT = relu(W2^T h1 + b2) ----
            h1p = penc.tile([HID, B], f32, name="encp", tag="encp")
            for k in range(NKT):
                nc.tensor.matmul(
                    h1p, lhsT=w1_sb[k], rhs=xt_sb[k],
                    start=(k == 0), stop=(k == NKT - 1),
                )
            # split relu1 across ACT / DVE so h2's matmul starts sooner
            h1bf = setup.tile([HID, B], bf16)
            S1 = 288
            nc.scalar.activation(h1bf[:, 0:S1], h1p[:, 0:S1], Act.Relu, bias=b1_sb)
            nc.vector.tensor_scalar(
                h1bf[:, S1:B], h1p[:, S1:B], b1_sb, 0.0, Alu.add, Alu.max
            )

            h2p = penc.tile([HID, B], f32, name="encp2", tag="encp")
            nc.tensor.matmul(h2p, lhsT=w2_sb, rhs=h1bf, start=True, stop=True)
            # hbf (bf16) split ACT/DVE; hT needed only for this core's 64 rows
            hbf = setup.tile([HID, B], bf16)
            nc.scalar.activation(hbf[:, 0:S1], h2p[:, 0:S1], Act.Relu, bias=b2_sb)
            nc.vector.tensor_scalar(
                hbf[:, S1:B], h2p[:, S1:B], b2_sb, 0.0, Alu.add, Alu.max
            )
            hT = setup.tile([HID, RPC], f32)
            nc.vector.tensor_scalar(
                hT, h2p[:, 0:RPC], b2_sb, 0.0, Alu.add, Alu.max
            )

            # ---- a2 = wp1a^T h + bp1, only for this core's 64 rows ----
            a2p = penc.tile([HID, RPC], f32, name="encp3", tag="encp")
            nc.tensor.matmul(a2p, lhsT=wp1a_sb, rhs=hbf[:, 0:RPC], start=True, stop=True)
            a2 = setup.tile([HID, RPC], f32)
            nc.vector.tensor_scalar(a2, a2p, bp1_sb, None, Alu.add)

            def emit_min(j, dtiles):
                if j in dtiles or j >= RPC:
                    return
                d = work.tile([HID, B], bf16, name="dtile")
                nc.vector.tensor_scalar(
                    d, hbf, hT[:, j : j + 1], None, Alu.min
                )
                dtiles[j] = d

            dtiles = {}
            emit_min(0, dtiles)
            emit_min(1, dtiles)

            # ---- pairwise main loop over this core's 64 rows ----
            # output accumulates in two [32,512] PSUM chunks; chunk A's
            # bias-add + DMA run mid-loop.
            outp = [pout.tile([HR, B], f32, name=f"outp{c}") for c in range(2)]
            outs = [setup.tile([HR, B], f32, name=f"outs{c}") for c in range(2)]
            pending = {}

            def emit_out(j):
                hid_j = pending.pop(j)
                c, jl = divmod(j, HR)
                nc.tensor.matmul(
                    outp[c],
                    lhsT=emb_sb[:, (c * HR + jl) * HR : (c * HR + jl + 1) * HR],
                    rhs=hid_j,
                    start=(jl == 0), stop=(jl == HR - 1),
                    skip_group_check=True,
                )
                if jl == HR - 1:
                    # chunk complete: bias-add + DMA out (chunk A mid-loop)
                    nc.vector.tensor_scalar(
                        outs[c], outp[c], bp2a_sb, None, Alu.add
                    )
                    eng = nc.gpsimd if c == 0 else nc.sync
                    eng.dma_start(
                        out=out_d[c * HR : (c + 1) * HR, :], in_=outs[c]
                    )

            for g in range((RPC + G - 1) // G):
                ils = [i for i in range(G * g, min(G * g + G, RPC))]
                for il in ils:
                    emit_min(il + G, dtiles)
                pps = []
                for il in ils:
                    pp = ppair.tile([HID, B], f32, name="pp")
                    nc.tensor.matmul(
                        pp, lhsT=w2p_sb, rhs=hbf,
                        start=True, stop=False, skip_group_check=True,
                    )
                    pps.append(pp)
                for il, pp in zip(ils, pps):
                    nc.tensor.matmul(
                        pp, lhsT=w3_sb, rhs=dtiles.pop(il),
                        start=False, stop=True, skip_group_check=True,
                    )
                for il, pp in zip(ils, pps):
                    hid = work.tile([HID, B], bf16, name="hid")
                    nc.scalar.activation(
                        hid[:, 0:SPLIT], pp[:, 0:SPLIT], Act.Relu,
                        bias=a2[:, il : il + 1],
                    )
                    nc.vector.tensor_scalar(
                        hid[:, SPLIT:B], pp[:, SPLIT:B],
                        a2[:, il : il + 1], 0.0, Alu.add, Alu.max,
                    )
                    pending[il] = hid
                    if il >= DEFER:
                        emit_out(il - DEFER)
            for j in range(RPC - DEFER, RPC):
                emit_out(j)

    nc.finalize()
    return nc


def _get_program():
    if "nc" not in _PROGRAM_CACHE:
        _PROGRAM_CACHE["nc"] = _build_program()
    return _PROGRAM_CACHE["nc"]


def _make_in_maps(x, W1, b1, W2, b2, Wp1, bp1, Wp2, bp2):
    bf16 = ml_dtypes.bfloat16
    f32 = np.float32
    x = np.asarray(x, dtype=f32)
    W1 = np.asarray(W1, dtype=f32)
    W2 = np.asarray(W2, dtype=f32)
    Wp1 = np.asarray(Wp1, dtype=f32)
    Wp2 = np.asarray(Wp2, dtype=f32).reshape(HID, 1)
    b1c = np.ascontiguousarray(np.asarray(b1, dtype=f32).reshape(HID, 1))
    b2c = np.ascontiguousarray(np.asarray(b2, dtype=f32).reshape(HID, 1))
    bp1c = np.ascontiguousarray(np.asarray(bp1, dtype=f32).reshape(HID, 1))
    bp2c = np.full((HR, 1), np.asarray(bp2, dtype=f32).reshape(-1)[0], dtype=f32)

    # |h_i - h_j| = h_i + h_j - 2*min(h_i, h_j) folds (see module docstring)
    w3f = Wp1[2 * HID : 3 * HID, :]
    wp1a = Wp1[0:HID, :] + w3f
    w2p = Wp1[HID : 2 * HID, :] + w3f
    w3 = -2.0 * w3f

    KPAD = NKT * HID  # 640: in_dim padded so every k-tile is 128 partitions

    # Wp2 embedded in two 32-row chunks: emb[:, c, j, m] = Wp2 if m == j
    emb = np.zeros((HID, 2, HR, HR), dtype=f32)
    idx = np.arange(HR)
    emb[:, 0, idx, idx] = Wp2
    emb[:, 1, idx, idx] = Wp2
    emb = np.ascontiguousarray(emb.reshape(HID, 2 * HR * HR)).astype(bf16)

    # packed weights [w2 | wp1a | w2p | w3] and biases [b1 | b2 | bp1 | bp2col]
    wpack = np.concatenate([W2, wp1a, w2p, w3], axis=1).astype(bf16)
    biases = np.zeros((HID, 4), dtype=f32)
    biases[:, 0:1] = b1c
    biases[:, 1:2] = b2c
    biases[:, 2:3] = bp1c
    biases[0:HR, 3:4] = bp2c

    # w1 padded to [640, 128], viewed as [128, 5*128]
    w1_pad = np.zeros((KPAD, HID), dtype=f32)
    w1_pad[:IN_DIM] = np.asarray(W1, dtype=f32)
    w1p = np.ascontiguousarray(
        w1_pad.reshape(NKT, HID, HID).transpose(1, 0, 2).reshape(HID, NKT * HID)
    ).astype(bf16)

    shared = dict(w1p=w1p, wpack=wpack, biases=biases, emb=emb)
    in_maps = []
    for c in range(NCORES):
        xr = np.roll(x, -c * RPC, axis=0)
        xt_pad = np.zeros((KPAD, B), dtype=f32)
        xt_pad[:IN_DIM] = xr.T
        xtp = np.ascontiguousarray(
            xt_pad.reshape(NKT, HID, B).transpose(1, 0, 2).reshape(HID, NKT * B)
        ).astype(bf16)
        m = dict(shared)
        m["xtp"] = xtp
        in_maps.append(m)
    return in_maps


def _run(in_maps, trace=False):
    from concourse.bass_utils import run_bass_kernel_spmd

    nc = _get_program()
    return run_bass_kernel_spmd(
        nc, in_maps, core_ids=list(range(NCORES)), trace=trace
    )


def kernel(x, W1, b1, W2, b2, Wp1, bp1, Wp2, bp2):
    in_maps = _make_in_maps(x, W1, b1, W2, b2, Wp1, bp1, Wp2, bp2)
    res = _run(in_maps, trace=False)
    out = np.empty((B, B), dtype=np.float32)
    for c in range(NCORES):
        blk = np.asarray(res.results[c]["out"], dtype=np.float32)
        out[c * RPC : (c + 1) * RPC, :] = np.roll(blk, c * RPC, axis=1)
    return out
